# revision 8
# baseline (speedup 1.0000x reference)
"""Trainium2 Bass kernel for nn_Encoder (dense MLP with stochastic ternarization).

y = tanh(x @ (s1*T(w1,n1)) + b1) @ (s2*T(w2,n2)) + b2,  T(w,n) = (w-n>1) - (w-n<-1)

Strategy: tensor-parallel over the 16384 hidden dim across 8 cores.
Each core gets a 2048-wide hidden shard of w1/noise1/s1/b1 (column-sharded) and
the matching 2048-row shard of w2/noise2. x is replicated (host pre-transposed
to bf16 xT so the contraction dim lands on SBUF partitions). Each core computes
partial yT = (h_shard @ w2_shard).T in fp32 per 512-wide batch block; a
ReduceScatter(add) per block over the 8 cores hands core c the summed rows
128c:128(c+1) of yT, where s2/b2 are applied. The host concatenates the 8
shards and transposes back.

Kernel structure (v2): all ternary weights are materialized in SBUF up front
(w1: 24 k-tiles of [128, 2048] bf16; w2: [128, 16, 1024] bf16), then a single
fused loop over 4 batch blocks runs layer1 (16 m-tiles) and layer2 (8 d-tiles)
back to back — h stays in SBUF, and each block's partial yT goes straight into
a chunked ReduceScatter that overlaps the next block's matmuls.

Ternarization on device: q = w - noise (DVE), then tanh(2^30*(q-1)) +
tanh(2^30*(q+1)) (ACT) which is exactly (q>1)-(q<-1) doubled, i.e. 2*T; the
extra factor 2 is folded into s1/s2 on the host (passed as 0.5*s).
"""

import sys

for _p in ("/opt/trn_rl_repo",):
    if _p not in sys.path:
        sys.path.insert(0, _p)

import numpy as np
import ml_dtypes

import concourse.bass as bass
import concourse.bacc as bacc
import concourse.mybir as mybir
import concourse.tile as tile
from concourse.bass_utils import run_bass_kernel_spmd

BF16 = mybir.dt.bfloat16
F32 = mybir.dt.float32
NPBF16 = ml_dtypes.bfloat16

N_CORES = 8
B = 2048          # batch
DIN = 3072        # input dim
DHID = 16384      # hidden dim
DOUT = 1024       # output dim
HSH = DHID // N_CORES   # 2048 hidden per core
DSH = DOUT // N_CORES   # 128 output rows per core after reduce-scatter

K1 = DIN // 128          # 24 contraction tiles, layer 1
K2 = HSH // 128          # 16 contraction tiles, layer 2
NB = B // 512            # 4 batch blocks of 512
MT = HSH // 128          # 16 hidden m-tiles
ND = DOUT // 128         # 8 dout tiles
QW = 512                 # ternarize stripe width for w1
NQ = HSH // QW           # 4 quarters

BIGK = float(2 ** 30)    # tanh(BIGK*(q -+ 1)) == sign(q -+ 1) exactly in fp32

TANH = mybir.ActivationFunctionType.Tanh
IDENT = mybir.ActivationFunctionType.Identity


def build_bass():
    nc = bacc.Bacc("TRN2", target_bir_lowering=False, debug=False, num_devices=N_CORES)

    xT = nc.dram_tensor("xT", [DIN, B], BF16, kind="ExternalInput")
    w1s = nc.dram_tensor("w1s", [DIN, HSH], F32, kind="ExternalInput")
    n1s = nc.dram_tensor("n1s", [DIN, HSH], F32, kind="ExternalInput")
    s1h = nc.dram_tensor("s1h", [128, MT], F32, kind="ExternalInput")
    b1m = nc.dram_tensor("b1m", [128, MT], F32, kind="ExternalInput")
    w2s = nc.dram_tensor("w2s", [HSH, DOUT], F32, kind="ExternalInput")
    n2s = nc.dram_tensor("n2s", [HSH, DOUT], F32, kind="ExternalInput")
    s2c = nc.dram_tensor("s2c", [128, 1], F32, kind="ExternalInput")
    b2c = nc.dram_tensor("b2c", [128, 1], F32, kind="ExternalInput")

    yTc = nc.dram_tensor("yTc", [DSH, B], F32, kind="ExternalOutput")

    with tile.TileContext(nc) as tc:
        with (
            tc.tile_pool(name="const", bufs=1) as cpool,
            tc.tile_pool(name="dram", bufs=1, space="DRAM") as dpool,
            tc.tile_pool(name="t2w1", bufs=K1) as t2pool,
            tc.tile_pool(name="t2w2", bufs=1) as t22pool,
            tc.tile_pool(name="stage", bufs=2) as spool,
            tc.tile_pool(name="xtn", bufs=1) as xpool,
            tc.tile_pool(name="hblk", bufs=16) as hpool,
            tc.tile_pool(name="yblk", bufs=3) as ypool,
            tc.tile_pool(name="fin", bufs=2) as fpool,
            tc.tile_pool(name="ps1", bufs=4, space="PSUM") as pspool,
            tc.tile_pool(name="ps2", bufs=4, space="PSUM") as ps2pool,
        ):
            s1_sb = cpool.tile([128, MT], F32, tag="s1")
            b1_sb = cpool.tile([128, MT], F32, tag="b1")
            s2_sb = cpool.tile([128, 1], F32, tag="s2")
            b2_sb = cpool.tile([128, 1], F32, tag="b2")
            nc.sync.dma_start(s1_sb[:], s1h[:, :])
            nc.sync.dma_start(b1_sb[:], b1m[:, :])
            nc.sync.dma_start(s2_sb[:], s2c[:, :])
            nc.sync.dma_start(b2_sb[:], b2c[:, :])
            kneg = cpool.tile([128, 1], F32, tag="kneg")
            nc.vector.memset(kneg[:], -BIGK)
            kpos = cpool.tile([128, 1], F32, tag="kpos")
            nc.vector.memset(kpos[:], BIGK)

            yT_n = [dpool.tile([DOUT, 512], F32, tag=f"yTp{n}", name=f"yT_n{n}") for n in range(NB)]
            rs_n = [dpool.tile([DSH, 512], F32, tag=f"rs{n}", name=f"rs_n{n}") for n in range(NB)]

            # ---- ternarize w1 into 24 resident k-tiles [128, 2048] bf16 ----
            # quarter-major order so the first m-tiles unblock as early as possible
            t2 = [t2pool.tile([128, HSH], BF16, tag="t2", name=f"t2_{k}") for k in range(K1)]

            def tern_stripe(dst_ap, src_w, src_n, r0, r1, c0, c1, fd):
                w_t = spool.tile([128, fd], F32, tag="w")
                nc.sync.dma_start(w_t[:], src_w[r0:r1, c0:c1])
                n_t = spool.tile([128, fd], F32, tag="n")
                nc.sync.dma_start(n_t[:], src_n[r0:r1, c0:c1])
                nc.vector.tensor_sub(w_t[:], w_t[:], n_t[:])
                a1 = spool.tile([128, fd], BF16, tag="a1")
                nc.scalar.activation(a1[:], w_t[:], TANH, bias=kneg[:, 0:1], scale=BIGK)
                a2 = spool.tile([128, fd], BF16, tag="a2")
                nc.scalar.activation(a2[:], w_t[:], TANH, bias=kpos[:, 0:1], scale=BIGK)
                nc.vector.tensor_add(dst_ap, a1[:], a2[:])

            for q in range(NQ):
                for k in range(K1):
                    tern_stripe(
                        t2[k][:, q * QW:(q + 1) * QW],
                        w1s, n1s, k * 128, (k + 1) * 128, q * QW, (q + 1) * QW, QW,
                    )
                if q == 0:
                    # w2 ternary right after the first quarter so layer2 of
                    # block 0 is never blocked on it
                    t22 = t22pool.tile([128, K2, DOUT], BF16, tag="t22")
                    for k2 in range(K2):
                        tern_stripe(
                            t22[:, k2, :],
                            w2s, n2s, k2 * 128, (k2 + 1) * 128, 0, DOUT, DOUT,
                        )

            # ---- fused per-batch-block layer1 + layer2 + chunked RS ----
            for n in range(NB):
                xtn = xpool.tile([128, K1, 512], BF16, tag="xtn")
                nc.sync.dma_start(
                    xtn[:],
                    xT[:, n * 512:(n + 1) * 512].rearrange("(k p) b -> p k b", p=128),
                )

                h_tiles = []
                for m in range(MT):
                    ps = pspool.tile([128, 512], F32, tag="ps")
                    for k in range(K1):
                        nc.tensor.matmul(
                            ps[:],
                            t2[k][:, m * 128:(m + 1) * 128],
                            xtn[:, k, :],
                            start=(k == 0),
                            stop=(k == K1 - 1),
                        )
                    h_m = hpool.tile([128, 512], BF16, tag="h")
                    nc.scalar.activation(
                        h_m[:], ps[:], TANH,
                        bias=b1_sb[:, m:m + 1],
                        scale=s1_sb[:, m:m + 1],
                    )
                    h_tiles.append(h_m)

                for d in range(ND):
                    ps2t = ps2pool.tile([128, 512], F32, tag="ps2")
                    for k2 in range(K2):
                        nc.tensor.matmul(
                            ps2t[:],
                            t22[:, k2, d * 128:(d + 1) * 128],
                            h_tiles[k2][:],
                            start=(k2 == 0),
                            stop=(k2 == K2 - 1),
                        )
                    y_sb = ypool.tile([128, 512], F32, tag="y")
                    nc.vector.tensor_copy(y_sb[:], ps2t[:])
                    nc.sync.dma_start(yT_n[n][d * 128:(d + 1) * 128, :], y_sb[:])

                nc.gpsimd.collective_compute(
                    "ReduceScatter",
                    mybir.AluOpType.add,
                    replica_groups=[list(range(N_CORES))],
                    ins=[yT_n[n].opt()],
                    outs=[rs_n[n].opt()],
                )
                rs_sb = fpool.tile([128, 512], F32, tag="rsb")
                nc.sync.dma_start(rs_sb[:], rs_n[n][:, :])
                out_sb = fpool.tile([128, 512], F32, tag="osb")
                nc.scalar.activation(
                    out_sb[:], rs_sb[:], IDENT,
                    bias=b2_sb[:, 0:1], scale=s2_sb[:, 0:1],
                )
                nc.sync.dma_start(yTc[:, n * 512:(n + 1) * 512], out_sb[:])

    nc.compile()
    return nc


_NC_CACHE = {}


def _get_nc():
    if "nc" not in _NC_CACHE:
        _NC_CACHE["nc"] = build_bass()
    return _NC_CACHE["nc"]


def _make_in_maps(x, w1, s1, b1, w2, s2, b2, noise1, noise2):
    x = np.asarray(x, dtype=np.float32)
    w1 = np.asarray(w1, dtype=np.float32)
    s1 = np.asarray(s1, dtype=np.float32)
    b1 = np.asarray(b1, dtype=np.float32)
    w2 = np.asarray(w2, dtype=np.float32)
    s2 = np.asarray(s2, dtype=np.float32)
    b2 = np.asarray(b2, dtype=np.float32)
    noise1 = np.asarray(noise1, dtype=np.float32)
    noise2 = np.asarray(noise2, dtype=np.float32)

    xT = np.ascontiguousarray(x.T).astype(NPBF16)
    in_maps = []
    for c in range(N_CORES):
        hs = slice(c * HSH, (c + 1) * HSH)
        ds = slice(c * DSH, (c + 1) * DSH)
        in_maps.append({
            "xT": xT,
            "w1s": np.ascontiguousarray(w1[:, hs]),
            "n1s": np.ascontiguousarray(noise1[:, hs]),
            "s1h": np.ascontiguousarray((0.5 * s1[hs]).reshape(MT, 128).T),
            "b1m": np.ascontiguousarray(b1[hs].reshape(MT, 128).T),
            "w2s": np.ascontiguousarray(w2[hs, :]),
            "n2s": np.ascontiguousarray(noise2[hs, :]),
            "s2c": np.ascontiguousarray((0.5 * s2[ds]).reshape(128, 1)),
            "b2c": np.ascontiguousarray(b2[ds].reshape(128, 1)),
        })
    return in_maps


def kernel(x, w1, s1, b1, w2, s2, b2, noise1, noise2, _bench_out=None):
    """Full-input, full-output entry point. Shards across 8 NeuronCores."""
    nc = _get_nc()
    in_maps = _make_in_maps(x, w1, s1, b1, w2, s2, b2, noise1, noise2)
    res = run_bass_kernel_spmd(nc, in_maps, core_ids=list(range(N_CORES)))
    if _bench_out is not None:
        _bench_out.append(res)
    yT = np.concatenate([res.results[c]["yTc"] for c in range(N_CORES)], axis=0)
    return np.ascontiguousarray(yT.T).astype(np.float32)


if __name__ == "__main__":
    nc = build_bass()
    print("built OK")
